# revision 2
# baseline (speedup 1.0000x reference)
"""MHA (1x1-conv qkv + attention over P with (d,t) features) on 8 trn2 cores.

Data-parallel over batch: core i handles batches [2i, 2i+2).  All matmul
moving operands are contiguous (strided rhs measured 3.8x slower) and all
psum->sbuf copies have >=32B inner runs (1KB-strided inner dims measured
3.7x slower); these two constraints drove the layout choices below.

Data-parallel over batch: core i handles batches [2i, 2i+2).

Per core, per batch (C=128 channels, H=2 heads, D=64, P=512, T=32):
  Phase A (per 128-row p-chunk): DMA x, cast fp16;
    - q/k projection: W^T stationary matmuls -> psum; ACT copies to
      q_sb/k_sb [c, t, p] fp16 (bias folded, q pre-scaled by D^-0.5).
    - v produced DIRECTLY TRANSPOSED: lhsT = x-chunk columns (stationary),
      rhs = Wv^T -> psum [p, (h d)] per t; DVE adds bias (broadcast tile)
      and casts to v_t [p', pb, t, (h d)] bf16.  No PE transposes.
  Phase B: dots_T[p',p] per head; the two heads' K=64 matmuls are issued
    as row-tiled pairs (head0 rows 0-63, head1 rows 64-127) so they run
    concurrently in disjoint PE subarrays; exp on psum -> attn bf16.
  Phase C (per 256-wide p half):
    - row sums via ones^T @ attn matmuls -> [1, p] psum directly;
      reciprocal; broadcast to [c, p] via a rank-1 K=1 matmul.
    - AV fused with the output transpose: out_t[(h d), p] = sum_{p'}
      v_t[p', (h d)]^T attn_T[p', p] per t, heads col-tiled into one
      psum bank; DVE multiplies by the broadcast reciprocal into
      of [c, p, t]; one fully-contiguous DMA per half to y.
"""

import numpy as np

import concourse.bass as bass
import concourse.tile as tile
from concourse import bacc, mybir
from concourse.bass_utils import run_bass_kernel_spmd
from concourse.masks import make_identity

B, C, P, T = 16, 128, 512, 32
H, D = 2, 64
SC = float(D) ** -0.5
NCORE = 8
BL = B // NCORE  # batches per core

F32 = mybir.dt.float32
F16 = mybir.dt.float16
BF16 = mybir.dt.bfloat16
Act = mybir.ActivationFunctionType


def build_nc(loop=1):
    nc = bacc.Bacc(None, target_bir_lowering=False)
    x_d = nc.dram_tensor("x", [BL, C, P, T], F32, kind="ExternalInput")
    w_d = nc.dram_tensor("W", [3 * C, C], F32, kind="ExternalInput")
    b_d = nc.dram_tensor("b", [3 * C], F32, kind="ExternalInput")
    y_d = nc.dram_tensor("y", [BL, C, P, T], F32, kind="ExternalOutput")

    with tile.TileContext(nc) as tc:
        with (
            tc.tile_pool(name="const", bufs=1) as constp,
            tc.tile_pool(name="xin", bufs=2) as xinp,
            tc.tile_pool(name="xc", bufs=2) as xcp,
            tc.tile_pool(name="qkv", bufs=1) as qkvp,
            tc.tile_pool(name="attn", bufs=1) as atp,
            tc.tile_pool(name="of", bufs=2) as ofp,
            tc.tile_pool(name="small", bufs=1) as smp,
            tc.tile_pool(name="p1", bufs=4, space="PSUM") as p1,
            tc.tile_pool(name="pav", bufs=2, space="PSUM") as pav,
            tc.tile_pool(name="psum_row", bufs=1, space="PSUM") as psr_pool,
            tc.tile_pool(name="pbc", bufs=1, space="PSUM") as pbc,
        ):
            # ---- constants ----
            id32 = constp.tile([128, 128], F32, tag="id32")
            make_identity(nc, id32[:, :])

            # W^T via PE transposes: wt[c, j, o] for j in (q, k, v), fp16
            wt = constp.tile([128, 3, 128], F16, tag="wt")
            for j in range(3):
                wraw = smp.tile([128, 128], F32, tag="wraw")
                nc.sync.dma_start(out=wraw[:, :], in_=w_d[j * 128:(j + 1) * 128, :])
                pw = p1.tile([128, 512], F32, tag="p1")
                nc.tensor.transpose(pw[:, 0:128], wraw[:, :], id32[:, :])
                nc.vector.tensor_copy(out=wt[:, j, :], in_=pw[:, 0:128])

            # bias: b[384] -> bcol[128, 3] (strided dma), bq pre-scaled
            bcol = constp.tile([128, 3], F32, tag="bcol")
            nc.sync.dma_start(out=bcol[:, :], in_=b_d[:].rearrange("(g c) -> c g", g=3))
            bqs = constp.tile([128, 1], F32, tag="bqs")
            nc.vector.tensor_scalar_mul(out=bqs[:, :], in0=bcol[:, 0:1], scalar1=SC)

            # ones
            ones_col = constp.tile([128, 1], BF16, tag="ones_col")
            nc.vector.memset(ones_col[:, :], 1.0)
            # bv broadcast tile [128, (4t x 128hd)] fp32: bv tiled 4x along free
            bvrow = constp.tile([1, 128], F32, tag="bvrow")
            nc.sync.dma_start(out=bvrow[0:1, :], in_=b_d[256:384])
            ones1 = constp.tile([1, 128], F32, tag="ones1")
            nc.vector.memset(ones1[:, :], 1.0)
            bv_bc = constp.tile([128, 512], F32, tag="bv_bc")
            pbv = pbc.tile([128, 512], F32, tag="pbc")
            for r in range(4):
                nc.tensor.matmul(
                    pbv[:, r * 128:(r + 1) * 128],
                    lhsT=ones1[0:1, :], rhs=bvrow[0:1, :],
                    start=True, stop=True, skip_group_check=True,
                )
            nc.vector.tensor_copy(out=bv_bc[:, :], in_=pbv[:, :])

            from contextlib import nullcontext
            with (tc.For_i(0, loop, name="rep") if loop else nullcontext()):
              for bi in range(BL):
                # [c, t, p] staging of q (fp16, pre-scaled+bias) and k (fp16)
                q_sb = qkvp.tile([128, T, P], F16, tag="q")
                k_sb = qkvp.tile([128, T, P], F16, tag="k")
                # v transposed: [p'(128 of block), block, t, (h d)] bf16
                v_t = qkvp.tile([128, 4, T, 128], BF16, tag="v")

                # ---- Phase A: projection (xc staged t-major) ----
                for pc in range(4):
                    xc = xcp.tile([128, T, 128], F16, tag="xc")
                    for hx in range(2):
                        xin = xinp.tile([128, 64, T], F32, tag="xin")
                        nc.sync.dma_start(
                            out=xin[:, :, :],
                            in_=x_d[bi, :, pc * 128 + hx * 64:pc * 128 + (hx + 1) * 64, :],
                        )
                        nc.vector.tensor_copy(
                            out=xc[:, :, hx * 64:(hx + 1) * 64],
                            in_=xin[:, :, :].transpose([0, 2, 1]),
                        )
                    # q/k/v interleaved per 4-t slice to spread copy engines
                    for s in range(8):
                        for j, dst in ((0, q_sb), (1, k_sb)):
                            rhs = xc[:, s * 4:(s + 1) * 4, :]
                            ps = p1.tile([128, 512], F32, tag="p1")
                            nc.tensor.matmul(
                                ps[:, :].rearrange("a (t p) -> a t p", t=4),
                                lhsT=wt[:, j, :],
                                rhs=rhs,
                                start=True,
                                stop=True,
                            )
                            out_ap = dst[:, s * 4:(s + 1) * 4,
                                         pc * 128:(pc + 1) * 128]
                            psv = ps[:, :].rearrange("a (t p) -> a t p", t=4)
                            if j == 0:
                                nc.scalar.activation(
                                    out_ap, psv, Act.Identity,
                                    bias=bqs[:, 0:1], scale=SC,
                                )
                            else:
                                nc.scalar.activation(
                                    out_ap, psv, Act.Identity,
                                    bias=bcol[:, 1:2], scale=1.0,
                                )
                        psv2 = p1.tile([128, 512], F32, tag="p1")
                        for dt in range(4):
                            t = s * 4 + dt
                            nc.tensor.matmul(
                                psv2[:, dt * 128:(dt + 1) * 128],
                                lhsT=xc[:, t, :],
                                rhs=wt[:, 2, :],
                                start=True,
                                stop=True,
                                skip_group_check=True,
                            )
                        nc.vector.scalar_tensor_tensor(
                            out=v_t[:, pc, s * 4:(s + 1) * 4, :],
                            in0=psv2[:, :].rearrange("a (t e) -> a t e", t=4),
                            scalar=0.0,
                            in1=bv_bc[:, :].rearrange("a (t e) -> a t e", t=4),
                            op0=mybir.AluOpType.add,
                            op1=mybir.AluOpType.add,
                        )

                # ---- Phase B: dots_T + exp, heads row-tiled in parallel ----
                attn0 = atp.tile([128, 4, P], BF16, tag="attn0")
                attn1 = atp.tile([128, 4, P], BF16, tag="attn1")
                for pb in range(4):
                    psd0 = p1.tile([128, 512], F32, tag="p1")
                    psd1 = p1.tile([128, 512], F32, tag="p1")
                    for t in range(T):
                        nc.tensor.matmul(
                            psd0[:, :],
                            lhsT=k_sb[0:64, t, pb * 128:(pb + 1) * 128],
                            rhs=q_sb[0:64, t, :],
                            start=(t == 0),
                            stop=(t == T - 1),
                        )
                        nc.tensor.matmul(
                            psd1[:, :],
                            lhsT=k_sb[64:128, t, pb * 128:(pb + 1) * 128],
                            rhs=q_sb[64:128, t, :],
                            start=(t == 0),
                            stop=(t == T - 1),
                        )
                    nc.scalar.activation(attn0[:, pb, :], psd0[:, :], Act.Exp)
                    nc.scalar.activation(attn1[:, pb, :], psd1[:, :], Act.Exp)

                # ---- Phase C: sums, reciprocal broadcast, AV, output ----
                for hf in range(2):
                    cs = slice(hf * 256, (hf + 1) * 256)
                    # row sums directly in row orientation: [1, p]
                    psrow = psr_pool.tile([1, 512], F32, tag="psr")
                    for h, attn_h in ((0, attn0), (1, attn1)):
                        for k4 in range(4):
                            nc.tensor.matmul(
                                psrow[0:1, h * 256:(h + 1) * 256],
                                lhsT=ones_col[:, :],
                                rhs=attn_h[:, k4, cs],
                                start=(k4 == 0),
                                stop=(k4 == 3),
                                skip_group_check=True,
                            )
                    rrow = smp.tile([1, 512], F32, tag="rrow")
                    nc.vector.reciprocal(rrow[0:1, :], psrow[0:1, :])
                    # broadcast 1/sum across channel partitions (rank-1 matmul)
                    psb = pbc.tile([128, 512], F32, tag="pbc")
                    for h in range(2):
                        nc.tensor.matmul(
                            psb[h * 64:(h + 1) * 64, 0:256],
                            lhsT=ones1[0:1, 0:64],
                            rhs=rrow[0:1, h * 256:(h + 1) * 256],
                            start=True,
                            stop=True,
                            skip_group_check=True,
                        )
                    rbc = smp.tile([128, 256], F32, tag="rbc")
                    nc.vector.tensor_copy(out=rbc[:, :], in_=psb[:, 0:256])

                    # AV + fused output transpose: out_t[(h d), p] per t
                    of = ofp.tile([128, 256, T], F32, tag="of")
                    for t in range(T):
                        pso = pav.tile([128, 512], F32, tag="pav")
                        for k4 in range(4):
                            nc.tensor.matmul(
                                pso[0:64, 0:256],
                                lhsT=v_t[:, k4, t, 0:64],
                                rhs=attn0[:, k4, cs],
                                start=(k4 == 0),
                                stop=(k4 == 3),
                                skip_group_check=True,
                            )
                            nc.tensor.matmul(
                                pso[64:128, 0:256],
                                lhsT=v_t[:, k4, t, 64:128],
                                rhs=attn1[:, k4, cs],
                                start=(k4 == 0),
                                stop=(k4 == 3),
                                skip_group_check=True,
                            )
                        nc.vector.scalar_tensor_tensor(
                            out=of[:, :, t],
                            in0=pso[:, 0:256],
                            scalar=0.0,
                            in1=rbc[:, :],
                            op0=mybir.AluOpType.add,
                            op1=mybir.AluOpType.mult,
                        )
                    nc.sync.dma_start(
                        out=y_d[bi, :, hf * 256:(hf + 1) * 256, :],
                        in_=of[:, :, :],
                    )
    if not nc.is_finalized():
        nc.finalize()
    return nc


_NC = None


def _get_nc():
    global _NC
    if _NC is None:
        _NC = build_nc(0)
    return _NC


def kernel(x, W, b):
    x = np.ascontiguousarray(x, dtype=np.float32)
    W = np.ascontiguousarray(W, dtype=np.float32)
    b = np.ascontiguousarray(b, dtype=np.float32)
    nc = _get_nc()
    in_maps = [
        {"x": x[i * BL:(i + 1) * BL], "W": W, "b": b} for i in range(NCORE)
    ]
    res = run_bass_kernel_spmd(nc, in_maps, list(range(NCORE)))
    out = np.concatenate([res.results[i]["y"] for i in range(NCORE)], axis=0)
    return out


if __name__ == "__main__":
    rng = np.random.default_rng(0)
    x = rng.standard_normal((B, C, P, T), dtype=np.float32)
    W = rng.standard_normal((3 * C, C), dtype=np.float32) * C ** -0.5
    b = rng.standard_normal(3 * C).astype(np.float32) * 0.01
    y = kernel(x=x, W=W, b=b)
    print(y.shape, y.dtype)
